# revision 9
# baseline (speedup 1.0000x reference)
"""HGNN extractor (2-layer hetero-GAT) Trainium2 Bass kernel, 8-core SPMD.

Strategy (edge-parallel via dst-range sharding):
- Host: sort edges by dst, shard dst ranges across 8 cores, 128-dst windows,
  per-window tile padding (tiles-per-window = max over cores so the SPMD
  program structure is identical on every core).
- Softmax ai[dst] term cancels within a dst segment -> attention weight is
  exp(aj[src]) only; segment max skipped (scores are O(1), exp safe; the
  1e-16 eps denominator difference is ~1e-13 relative, below fp32 noise).
- Aggregation via one-hot matmul scatter into PSUM per 128-dst window.
- d1 (op->mc): host pre-gathers 4-dim raw src features (transposed);
  projections recomputed on device (gather volume 16B/edge).
- AllGather of updated h_mc between directions (the only collective).
- d2 (mc->op): device indirect-DMA gathers updated h_mc rows, PE-transpose.
"""
import os
import sys

import numpy as np

for _p in ("/opt/trn_rl_repo", "/root/.axon_site/_ro/trn_rl_repo"):
    if os.path.isdir(_p) and _p not in sys.path:
        sys.path.append(_p)

from contextlib import ExitStack

import concourse.bass as bass
import concourse.bacc as bacc
import concourse.tile as tile
from concourse import mybir
from concourse.bass_utils import run_bass_kernel_spmd
from concourse.masks import make_identity

F32 = mybir.dt.float32
F32R = mybir.dt.float32r
I32 = mybir.dt.int32
AF = mybir.ActivationFunctionType
OP = mybir.AluOpType

HEADS, EMBED = 8, 64
N_CORES = 8
EPS = 1e-16
P = 128

LAST_EXEC_NS = None  # set by kernel() when tracing


# ---------------------------------------------------------------- host prep
def build_dir_plan(src, dst, n_dst, feats=None):
    order = np.argsort(dst, kind="stable")
    src_s, dst_s = src[order], dst[order]
    Dk = n_dst // N_CORES
    n_win = (Dk + P - 1) // P
    core_lo = np.searchsorted(dst_s, np.arange(N_CORES) * Dk)
    core_hi = np.searchsorted(dst_s, (np.arange(N_CORES) + 1) * Dk)
    T = np.zeros(n_win, dtype=np.int64)
    win_edges = []
    for k in range(N_CORES):
        lo_k, hi_k = core_lo[k], core_hi[k]
        dloc = dst_s[lo_k:hi_k] - k * Dk
        rows = []
        for w in range(n_win):
            a = lo_k + np.searchsorted(dloc, w * P)
            b = lo_k + np.searchsorted(dloc, (w + 1) * P)
            rows.append((a, b))
            T[w] = max(T[w], (b - a + P - 1) // P)
        win_edges.append(rows)
    T = np.maximum(T, 1)
    tiles_total = int(T.sum())
    E_pad = tiles_total * P
    dstrel = np.full((N_CORES, E_pad), -1.0, dtype=np.float32)
    srcidx = np.zeros((N_CORES, E_pad), dtype=np.int32)
    featT = None
    if feats is not None:
        F = feats.shape[1]
        featT = np.zeros((N_CORES, F, E_pad), dtype=np.float32)
    ofs = np.concatenate([[0], np.cumsum(T)])
    for k in range(N_CORES):
        for w in range(n_win):
            a, b = win_edges[k][w]
            n = b - a
            o = int(ofs[w]) * P
            dstrel[k, o:o + n] = (dst_s[a:b] - k * Dk - w * P).astype(np.float32)
            srcidx[k, o:o + n] = src_s[a:b].astype(np.int32)
            if feats is not None:
                featT[k, :, o:o + n] = feats[src_s[a:b]].T
    win_meta = [(int(ofs[w]), int(ofs[w + 1]), w * P, min(P, Dk - w * P))
                for w in range(n_win)]
    # column-major [128, tiles] views for per-window single-DMA loads
    dstrel_c = dstrel.reshape(N_CORES, tiles_total, P).transpose(0, 2, 1).copy()
    srcidx_c = srcidx.reshape(N_CORES, tiles_total, P).transpose(0, 2, 1).copy()
    return dict(dstrel_c=dstrel_c, srcidx_c=srcidx_c, featT=featT,
                win_meta=win_meta, Dk=Dk, n_win=n_win, tiles_total=tiles_total,
                E_pad=E_pad)


def _vjT(att, W):
    att_j = att[0, :, EMBED:]
    return np.stack([att_j[h] @ W[h * EMBED:(h + 1) * EMBED, :]
                     for h in range(HEADS)], axis=1).astype(np.float32)


# ------------------------------------------------------------- device build
def _elu(nc, pool, out_ap, in_ap, tag):
    """out = elu(in): x - min(x,0) + exp(min(x,0)) - 1. 5 ops."""
    shp = list(in_ap.shape)
    m = pool.tile(shp, F32, tag=tag + "m")
    e = pool.tile(shp, F32, tag=tag + "e")
    nc.vector.tensor_scalar_min(m[:], in_ap, 0.0)
    nc.scalar.activation(e[:], m[:], AF.Exp)
    nc.vector.tensor_tensor(out_ap, in_ap, m[:], op=OP.subtract)
    nc.vector.tensor_tensor(out_ap, out_ap, e[:], op=OP.add)
    nc.vector.tensor_scalar_add(out_ap, out_ap, -1.0)


def _residual_precompute(nc, sb, psA, psB, raw_dram, wT_sb, F, n_tiles, out_dram,
                         ident, tag):
    """h = elu(raw @ W.T) for node rows, tile by tile of 128."""
    for t in range(n_tiles):
        r = sb.tile([P, F], F32, tag=tag + "r")
        nc.sync.dma_start(r[:], raw_dram[t * P:(t + 1) * P, :])
        pt = psB.tile([F, P], F32, tag="aux")
        nc.tensor.transpose(out=pt[:], in_=r[:], identity=ident[:])
        rT = sb.tile([F, P], F32, tag=tag + "rT")
        nc.vector.tensor_copy(rT[:], pt[:])
        ph = psA.tile([P, EMBED], F32, tag="hs")
        nc.tensor.matmul(ph[:], lhsT=rT[:], rhs=wT_sb[:], start=True, stop=True)
        o = sb.tile([P, EMBED], F32, tag=tag + "o")
        _elu(nc, sb, o[:], ph[:], tag + "el")
        nc.sync.dma_start(out_dram[t * P:(t + 1) * P, :], o[:])


def _gat_direction(nc, tc, sb, psA, psB, plan, *, d1_featT, srcidx_dram,
                   gather_src_dram, dstrel_dram, WT_sb, vj_sb, res_dram,
                   out_drams, ident, iota, use_f32r, tag):
    """Windowed GAT direction. d1: d1_featT set (pre-gathered raw features,
    projected via W_op on device). d2: srcidx/gather_src set (indirect gather
    of updated h_mc rows + PE transpose)."""
    win_meta = plan["win_meta"]
    cast = (lambda ap: ap.bitcast(F32R)) if use_f32r else (lambda ap: ap)
    for (t_lo, t_hi, base, rows) in win_meta:
        T_w = t_hi - t_lo
        drw = sb.tile([P, T_w], F32, tag=tag + "dr")
        nc.sync.dma_start(drw[:], dstrel_dram[:, t_lo:t_hi])
        pM = psA.tile([P, 512], F32, tag="m4M")
        pW = psA.tile([P, HEADS], F32, tag="m4W")

        if d1_featT is not None:
            # chunks of up to 4 tiles: h_srcT = elu(W_op @ featT_chunk)
            hT_chunks = []
            for c_lo in range(t_lo, t_hi, 4):
                n_t = min(4, t_hi - c_lo)
                wdt = n_t * P
                f = sb.tile([4, 512], F32, tag=tag + "f")
                nc.sync.dma_start(f[:, :wdt],
                                  d1_featT[:, c_lo * P:c_lo * P + wdt])
                phc = psB.tile([EMBED, 512], F32, tag="aux")
                nc.tensor.matmul(phc[:, :wdt], lhsT=cast(WT_sb["W_feat"][:]),
                                 rhs=cast(f[:4, :wdt]), start=True, stop=True)
                hTc = sb.tile([EMBED, 512], F32, tag=tag + "hT")
                _elu(nc, sb, hTc[:, :wdt], phc[:, :wdt], tag + "el")
                hT_chunks.append((c_lo, hTc))
        else:
            idxw = sb.tile([P, T_w], I32, tag=tag + "ix")
            nc.sync.dma_start(idxw[:], srcidx_dram[:, t_lo:t_hi])

        for t in range(t_lo, t_hi):
            if d1_featT is not None:
                c_lo, hTc = hT_chunks[(t - t_lo) // 4]
                hT = hTc[:, (t - c_lo) * P:(t - c_lo + 1) * P]
            else:
                g = sb.tile([P, EMBED], F32, tag=tag + "g")
                nc.gpsimd.indirect_dma_start(
                    out=g[:, :], out_offset=None,
                    in_=gather_src_dram[:, :],
                    in_offset=bass.IndirectOffsetOnAxis(
                        ap=idxw[:, t - t_lo:t - t_lo + 1], axis=0),
                )
                pt = psB.tile([EMBED, P], F32, tag="aux")
                nc.tensor.transpose(out=pt[:], in_=g[:], identity=ident[:])
                hT2 = sb.tile([EMBED, P], F32, tag=tag + "hT2")
                nc.vector.tensor_copy(hT2[:], pt[:])
                hT = hT2[:, :]

            phs = psA.tile([P, 512], F32, tag="hs")
            nc.tensor.matmul(phs[:], lhsT=cast(hT), rhs=cast(WT_sb["W_big"][:]),
                             start=True, stop=True)
            paj = psB.tile([P, HEADS], F32, tag="aj")
            nc.tensor.matmul(paj[:], lhsT=hT, rhs=vj_sb[:],
                             start=True, stop=True)
            w = sb.tile([P, HEADS], F32, tag=tag + "w")
            nc.scalar.activation(w[:], paj[:], AF.Exp)
            msg = sb.tile([P, 512], F32, tag=tag + "ms")
            nc.vector.tensor_tensor(
                msg[:].rearrange("p (h c) -> p h c", h=HEADS),
                phs[:].rearrange("p (h c) -> p h c", h=HEADS),
                w[:].to_broadcast([P, HEADS, EMBED]),
                op=OP.mult)
            S = sb.tile([P, P], F32, tag=tag + "S")
            nc.vector.tensor_tensor(S[:], drw[:, t - t_lo:t - t_lo + 1]
                                    .to_broadcast([P, P]), iota[:],
                                    op=OP.is_equal)
            nc.tensor.matmul(pM[:], lhsT=cast(S[:]), rhs=cast(msg[:]),
                             start=(t == t_lo), stop=(t == t_hi - 1))
            nc.tensor.matmul(pW[:], lhsT=S[:], rhs=w[:],
                             start=(t == t_lo), stop=(t == t_hi - 1))

        # finalize window: out = elu(res + mean_h(M/(W+eps)))
        rec = sb.tile([P, HEADS], F32, tag=tag + "rc")
        nc.vector.tensor_scalar_add(rec[:], pW[:], EPS)
        nc.vector.reciprocal(rec[:], rec[:])
        t1 = sb.tile([P, 512], F32, tag=tag + "t1")
        nc.vector.tensor_tensor(
            t1[:].rearrange("p (h c) -> p h c", h=HEADS),
            pM[:].rearrange("p (h c) -> p h c", h=HEADS),
            rec[:].to_broadcast([P, HEADS, EMBED]), op=OP.mult)
        s2 = sb.tile([P, 256], F32, tag=tag + "s2")
        nc.vector.tensor_tensor(s2[:], t1[:, 0:256], t1[:, 256:512], op=OP.add)
        s3 = sb.tile([P, 128], F32, tag=tag + "s3")
        nc.vector.tensor_tensor(s3[:], s2[:, 0:128], s2[:, 128:256], op=OP.add)
        s4 = sb.tile([P, 64], F32, tag=tag + "s4")
        nc.vector.tensor_tensor(s4[:], s3[:, 0:64], s3[:, 64:128], op=OP.add)
        res = sb.tile([P, EMBED], F32, tag=tag + "re")
        nc.sync.dma_start(res[:], res_dram[base:base + P, :])
        x = sb.tile([P, EMBED], F32, tag=tag + "x")
        nc.vector.tensor_scalar(x[:], s4[:], 0.125, None, op0=OP.mult)
        nc.vector.tensor_tensor(x[:], x[:], res[:], op=OP.add)
        o = sb.tile([P, EMBED], F32, tag=tag + "o")
        _elu(nc, sb, o[:], x[:], tag + "fe")
        for od in out_drams:
            nc.sync.dma_start(od[base:base + rows, :], o[:rows, :])


def build_program(p1, p2, use_f32r=True):
    nc = bacc.Bacc("TRN2", target_bir_lowering=False, num_devices=N_CORES)
    E1, E2 = p1["E_pad"], p2["E_pad"]
    nt1, nt2 = p1["tiles_total"], p2["tiles_total"]
    Dk1, Dk2 = p1["Dk"], p2["Dk"]
    nw1, nw2 = p1["n_win"], p2["n_win"]

    din = dict(
        d1_featT=nc.dram_tensor("d1_featT", [4, E1], F32, kind="ExternalInput"),
        d1_dstrel=nc.dram_tensor("d1_dstrel", [P, nt1], F32, kind="ExternalInput"),
        d2_srcidx=nc.dram_tensor("d2_srcidx", [P, nt2], I32, kind="ExternalInput"),
        d2_dstrel=nc.dram_tensor("d2_dstrel", [P, nt2], F32, kind="ExternalInput"),
        mc_slice=nc.dram_tensor("mc_slice", [nw1 * P, 2], F32, kind="ExternalInput"),
        op_slice=nc.dram_tensor("op_slice", [nw2 * P, 4], F32, kind="ExternalInput"),
        W_opT=nc.dram_tensor("W_opT", [4, EMBED], F32, kind="ExternalInput"),
        W_mcT=nc.dram_tensor("W_mcT", [2, EMBED], F32, kind="ExternalInput"),
        W_omT=nc.dram_tensor("W_omT", [EMBED, 512], F32, kind="ExternalInput"),
        W_moT=nc.dram_tensor("W_moT", [EMBED, 512], F32, kind="ExternalInput"),
        vj1=nc.dram_tensor("vj1", [EMBED, HEADS], F32, kind="ExternalInput"),
        vj2=nc.dram_tensor("vj2", [EMBED, HEADS], F32, kind="ExternalInput"),
        iota=nc.dram_tensor("iota", [P, P], F32, kind="ExternalInput"),
    )
    h_mc_out = nc.dram_tensor("h_mc_out", [Dk1, EMBED], F32, kind="ExternalOutput")
    h_op_out = nc.dram_tensor("h_op_out", [Dk2, EMBED], F32, kind="ExternalOutput")
    h_mc_res = nc.dram_tensor("h_mc_res", [nw1 * P, EMBED], F32, kind="Internal")
    h_op_res = nc.dram_tensor("h_op_res", [nw2 * P, EMBED], F32, kind="Internal")
    cc_in = nc.dram_tensor("cc_in", [Dk1, EMBED], F32, kind="Internal")
    cc_out = nc.dram_tensor("cc_out", [Dk1 * N_CORES, EMBED], F32,
                            kind="Internal", addr_space="Shared")

    with ExitStack() as ctx:
        tc = ctx.enter_context(tile.TileContext(nc))
        sb = ctx.enter_context(tc.tile_pool(name="sb", bufs=3))
        wpool = ctx.enter_context(tc.tile_pool(name="wp", bufs=1))
        psA = ctx.enter_context(tc.tile_pool(name="psA", bufs=2, space="PSUM"))
        psB = ctx.enter_context(tc.tile_pool(name="psB", bufs=1, space="PSUM"))

        ident = wpool.tile([P, P], F32, tag="ident")
        make_identity(nc, ident[:])
        iota = wpool.tile([P, P], F32, tag="iota")
        nc.sync.dma_start(iota[:], din["iota"][:, :])

        def _load_const(name, shape):
            t = wpool.tile(shape, F32, tag=name)
            nc.sync.dma_start(t[:], din[name][:, :])
            return t

        W_opT_sb = _load_const("W_opT", [4, EMBED])
        W_mcT_sb = _load_const("W_mcT", [2, EMBED])
        W_omT_sb = _load_const("W_omT", [EMBED, 512])
        W_moT_sb = _load_const("W_moT", [EMBED, 512])
        vj1_sb = _load_const("vj1", [EMBED, HEADS])
        vj2_sb = _load_const("vj2", [EMBED, HEADS])

        _residual_precompute(nc, sb, psA, psB, din["mc_slice"], W_mcT_sb, 2,
                             nw1, h_mc_res, ident, "rm")
        _residual_precompute(nc, sb, psA, psB, din["op_slice"], W_opT_sb, 4,
                             nw2, h_op_res, ident, "ro")

        _gat_direction(nc, tc, sb, psA, psB, p1,
                       d1_featT=din["d1_featT"], srcidx_dram=None,
                       gather_src_dram=None, dstrel_dram=din["d1_dstrel"],
                       WT_sb={"W_feat": W_opT_sb, "W_big": W_omT_sb},
                       vj_sb=vj1_sb, res_dram=h_mc_res,
                       out_drams=[h_mc_out, cc_in], ident=ident, iota=iota,
                       use_f32r=use_f32r, tag="a")

        tc.strict_bb_all_engine_barrier()
        nc.gpsimd.collective_compute(
            "AllGather", OP.bypass,
            replica_groups=[list(range(N_CORES))],
            ins=[cc_in[:, :]], outs=[cc_out[:, :]])
        tc.strict_bb_all_engine_barrier()

        _gat_direction(nc, tc, sb, psA, psB, p2,
                       d1_featT=None, srcidx_dram=din["d2_srcidx"],
                       gather_src_dram=cc_out, dstrel_dram=din["d2_dstrel"],
                       WT_sb={"W_big": W_moT_sb}, vj_sb=vj2_sb,
                       res_dram=h_op_res, out_drams=[h_op_out], ident=ident,
                       iota=iota, use_f32r=use_f32r, tag="b")
    if not nc.is_finalized():
        nc.finalize()
    return nc


# ------------------------------------------------------------------- driver
def kernel(op_nodes, mc_nodes, edge_index_om, W_op, b_op, W_mc, b_mc,
           W_om, att_om, W_mo, att_mo):
    global LAST_EXEC_NS
    op_nodes = np.asarray(op_nodes, dtype=np.float32)
    mc_nodes = np.asarray(mc_nodes, dtype=np.float32)
    ei = np.asarray(edge_index_om)
    src, dst = ei[0].astype(np.int64), ei[1].astype(np.int64)
    n_op, n_mc = op_nodes.shape[0], mc_nodes.shape[0]

    p1 = build_dir_plan(src, dst, n_mc, feats=op_nodes)
    p2 = build_dir_plan(dst, src, n_op)

    nc = build_program(p1, p2, use_f32r=False)

    iota = np.broadcast_to(np.arange(P, dtype=np.float32), (P, P)).copy()
    nw1, nw2 = p1["n_win"], p2["n_win"]
    in_maps = []
    for k in range(N_CORES):
        mc_sl = np.zeros((nw1 * P, 2), np.float32)
        mc_sl[:p1["Dk"]] = mc_nodes[k * p1["Dk"]:(k + 1) * p1["Dk"]]
        op_sl = np.zeros((nw2 * P, 4), np.float32)
        op_sl[:p2["Dk"]] = op_nodes[k * p2["Dk"]:(k + 1) * p2["Dk"]]
        in_maps.append(dict(
            d1_featT=p1["featT"][k],
            d1_dstrel=p1["dstrel_c"][k],
            d2_srcidx=p2["srcidx_c"][k],
            d2_dstrel=p2["dstrel_c"][k],
            mc_slice=mc_sl, op_slice=op_sl,
            W_opT=np.ascontiguousarray(W_op.T, dtype=np.float32),
            W_mcT=np.ascontiguousarray(W_mc.T, dtype=np.float32),
            W_omT=np.ascontiguousarray(W_om.T, dtype=np.float32),
            W_moT=np.ascontiguousarray(W_mo.T, dtype=np.float32),
            vj1=_vjT(np.asarray(att_om, np.float32), np.asarray(W_om, np.float32)),
            vj2=_vjT(np.asarray(att_mo, np.float32), np.asarray(W_mo, np.float32)),
            iota=iota,
        ))

    trace = bool(os.environ.get("KERNEL_TRACE"))
    try:
        res = run_bass_kernel_spmd(nc, in_maps, list(range(N_CORES)), trace=trace)
    except ModuleNotFoundError:
        res = run_bass_kernel_spmd(nc, in_maps, list(range(N_CORES)), trace=False)
    LAST_EXEC_NS = res.exec_time_ns
    if res.profile_json:
        with open("/root/problem/profile.json", "w") as f:
            f.write(res.profile_json)
    if os.environ.get("KERNEL_TIME_RERUN"):
        import time as _time
        t0 = _time.time()
        run_bass_kernel_spmd(nc, in_maps, list(range(N_CORES)), trace=False)
        LAST_EXEC_NS = LAST_EXEC_NS or int((_time.time() - t0) * 1e9)
    h_mc = np.concatenate([r["h_mc_out"] for r in res.results], axis=0)
    h_op = np.concatenate([r["h_op_out"] for r in res.results], axis=0)
    return (h_op, h_mc)


# revision 10
# speedup vs baseline: 1.4100x; 1.4100x over previous
"""HGNN extractor (2-layer hetero-GAT) Trainium2 Bass kernel, 8-core SPMD.

Strategy (edge-parallel via dst-range sharding):
- Host: sort edges by dst, shard dst ranges across 8 cores, 128-dst windows,
  per-window tile padding (tiles-per-window = max over cores so the SPMD
  program structure is identical on every core).
- Softmax ai[dst] term cancels within a dst segment -> attention weight is
  exp(aj[src]) only; segment max skipped (scores are O(1), exp safe; the
  1e-16 eps denominator difference is ~1e-13 relative, below fp32 noise).
- Aggregation via one-hot matmul scatter into PSUM per 128-dst window.
- d1 (op->mc): host pre-gathers 4-dim raw src features (transposed);
  projections recomputed on device (gather volume 16B/edge).
- AllGather of updated h_mc between directions (the only collective).
- d2 (mc->op): device indirect-DMA gathers updated h_mc rows, PE-transpose.
"""
import os
import sys

import numpy as np

for _p in ("/opt/trn_rl_repo", "/root/.axon_site/_ro/trn_rl_repo"):
    if os.path.isdir(_p) and _p not in sys.path:
        sys.path.append(_p)

from contextlib import ExitStack

import concourse.bass as bass
import concourse.bacc as bacc
import concourse.tile as tile
from concourse import mybir
from concourse.bass_utils import run_bass_kernel_spmd
from concourse.masks import make_identity

F32 = mybir.dt.float32
F32R = mybir.dt.float32r
BF16 = mybir.dt.bfloat16
USE_BF16 = os.environ.get("KERNEL_FP32") is None
MMDT = BF16 if USE_BF16 else F32
I32 = mybir.dt.int32
AF = mybir.ActivationFunctionType
OP = mybir.AluOpType

HEADS, EMBED = 8, 64
N_CORES = 8
EPS = 1e-16
P = 128

LAST_EXEC_NS = None  # set by kernel() when tracing


# ---------------------------------------------------------------- host prep
def build_dir_plan(src, dst, n_dst, feats=None):
    order = np.argsort(dst, kind="stable")
    src_s, dst_s = src[order], dst[order]
    Dk = n_dst // N_CORES
    n_win = (Dk + P - 1) // P
    core_lo = np.searchsorted(dst_s, np.arange(N_CORES) * Dk)
    core_hi = np.searchsorted(dst_s, (np.arange(N_CORES) + 1) * Dk)
    T = np.zeros(n_win, dtype=np.int64)
    win_edges = []
    for k in range(N_CORES):
        lo_k, hi_k = core_lo[k], core_hi[k]
        dloc = dst_s[lo_k:hi_k] - k * Dk
        rows = []
        for w in range(n_win):
            a = lo_k + np.searchsorted(dloc, w * P)
            b = lo_k + np.searchsorted(dloc, (w + 1) * P)
            rows.append((a, b))
            T[w] = max(T[w], (b - a + P - 1) // P)
        win_edges.append(rows)
    T = np.maximum(T, 1)
    tiles_total = int(T.sum())
    E_pad = tiles_total * P
    dstrel = np.full((N_CORES, E_pad), -1.0, dtype=np.float32)
    srcidx = np.zeros((N_CORES, E_pad), dtype=np.int32)
    featT = None
    if feats is not None:
        F = feats.shape[1]
        featT = np.zeros((N_CORES, F, E_pad), dtype=np.float32)
    ofs = np.concatenate([[0], np.cumsum(T)])
    for k in range(N_CORES):
        for w in range(n_win):
            a, b = win_edges[k][w]
            n = b - a
            o = int(ofs[w]) * P
            dstrel[k, o:o + n] = (dst_s[a:b] - k * Dk - w * P).astype(np.float32)
            srcidx[k, o:o + n] = src_s[a:b].astype(np.int32)
            if feats is not None:
                featT[k, :, o:o + n] = feats[src_s[a:b]].T
    win_meta = [(int(ofs[w]), int(ofs[w + 1]), w * P, min(P, Dk - w * P))
                for w in range(n_win)]
    # column-major [128, tiles] views for per-window single-DMA loads
    dstrel_c = dstrel.reshape(N_CORES, tiles_total, P).transpose(0, 2, 1).copy()
    srcidx_c = srcidx.reshape(N_CORES, tiles_total, P).transpose(0, 2, 1).copy()
    return dict(dstrel_c=dstrel_c, srcidx_c=srcidx_c, featT=featT,
                win_meta=win_meta, Dk=Dk, n_win=n_win, tiles_total=tiles_total,
                E_pad=E_pad)


def _vjT(att, W):
    att_j = att[0, :, EMBED:]
    return np.stack([att_j[h] @ W[h * EMBED:(h + 1) * EMBED, :]
                     for h in range(HEADS)], axis=1).astype(np.float32)


# ------------------------------------------------------------- device build
def _elu(nc, pool, out_ap, in_ap, tag):
    """out = elu(in): x - min(x,0) + exp(min(x,0)) - 1. 5 ops."""
    shp = list(in_ap.shape)
    m = pool.tile(shp, F32, tag=tag + "m")
    e = pool.tile(shp, F32, tag=tag + "e")
    nc.vector.tensor_scalar_min(m[:], in_ap, 0.0)
    nc.scalar.activation(e[:], m[:], AF.Exp)
    nc.vector.tensor_tensor(out_ap, in_ap, m[:], op=OP.subtract)
    nc.vector.tensor_tensor(out_ap, out_ap, e[:], op=OP.add)
    nc.vector.tensor_scalar_add(out_ap, out_ap, -1.0)


def _residual_precompute(nc, sb, psA, psB, raw_dram, wT_sb, F, n_tiles, out_dram,
                         ident, tag):
    """h = elu(raw @ W.T) for node rows, tile by tile of 128."""
    for t in range(n_tiles):
        r = sb.tile([P, F], F32, tag=tag + "r")
        nc.sync.dma_start(r[:], raw_dram[t * P:(t + 1) * P, :])
        pt = psB.tile([F, P], F32, tag="aux")
        nc.tensor.transpose(out=pt[:], in_=r[:], identity=ident[:])
        rT = sb.tile([F, P], MMDT, tag=tag + "rT")
        nc.vector.tensor_copy(rT[:], pt[:])
        ph = psA.tile([P, EMBED], F32, tag="hs")
        nc.tensor.matmul(ph[:], lhsT=rT[:], rhs=wT_sb[:], start=True, stop=True)
        o = sb.tile([P, EMBED], F32, tag=tag + "o")
        _elu(nc, sb, o[:], ph[:], tag + "el")
        nc.sync.dma_start(out_dram[t * P:(t + 1) * P, :], o[:])


def _gat_direction(nc, tc, sb, psA, psB, plan, *, d1_featT, srcidx_dram,
                   gather_src_dram, dstrel_dram, WT_sb, vj_sb, res_dram,
                   out_drams, ident, iota, use_f32r, tag):
    """Windowed GAT direction. d1: d1_featT set (pre-gathered raw features,
    projected via W_op on device). d2: srcidx/gather_src set (indirect gather
    of updated h_mc rows + PE transpose)."""
    win_meta = plan["win_meta"]
    cast = (lambda ap: ap.bitcast(F32R)) if use_f32r else (lambda ap: ap)
    for (t_lo, t_hi, base, rows) in win_meta:
        T_w = t_hi - t_lo
        drw = sb.tile([P, T_w], F32, tag=tag + "dr")
        nc.sync.dma_start(drw[:], dstrel_dram[:, t_lo:t_hi])
        pM = psA.tile([P, 512], F32, tag="m4M")
        pW = psA.tile([P, HEADS], F32, tag="m4W")

        if d1_featT is not None:
            # chunks of up to 4 tiles: h_srcT = elu(W_op @ featT_chunk)
            hT_chunks = []
            for c_lo in range(t_lo, t_hi, 4):
                n_t = min(4, t_hi - c_lo)
                wdt = n_t * P
                f = sb.tile([4, 512], MMDT, tag=tag + "f")
                nc.sync.dma_start(f[:, :wdt],
                                  d1_featT[:, c_lo * P:c_lo * P + wdt])
                phc = psB.tile([EMBED, 512], F32, tag="aux")
                nc.tensor.matmul(phc[:, :wdt], lhsT=cast(WT_sb["W_feat"][:]),
                                 rhs=cast(f[:4, :wdt]), start=True, stop=True)
                hTc = sb.tile([EMBED, 512], MMDT, tag=tag + "hT")
                _elu(nc, sb, hTc[:, :wdt], phc[:, :wdt], tag + "el")
                hT_chunks.append((c_lo, hTc))
        else:
            idxw = sb.tile([P, T_w], I32, tag=tag + "ix")
            nc.sync.dma_start(idxw[:], srcidx_dram[:, t_lo:t_hi])

        for t in range(t_lo, t_hi):
            if d1_featT is not None:
                c_lo, hTc = hT_chunks[(t - t_lo) // 4]
                hT = hTc[:, (t - c_lo) * P:(t - c_lo + 1) * P]
            else:
                g = sb.tile([P, EMBED], F32, tag=tag + "g")
                nc.gpsimd.indirect_dma_start(
                    out=g[:, :], out_offset=None,
                    in_=gather_src_dram[:, :],
                    in_offset=bass.IndirectOffsetOnAxis(
                        ap=idxw[:, t - t_lo:t - t_lo + 1], axis=0),
                )
                pt = psB.tile([EMBED, P], F32, tag="aux")
                nc.tensor.transpose(out=pt[:], in_=g[:], identity=ident[:])
                hT2 = sb.tile([EMBED, P], MMDT, tag=tag + "hT2")
                nc.vector.tensor_copy(hT2[:], pt[:])
                hT = hT2[:, :]

            phs = psA.tile([P, 512], F32, tag="hs")
            nc.tensor.matmul(phs[:], lhsT=cast(hT), rhs=cast(WT_sb["W_big"][:]),
                             start=True, stop=True)
            paj = psB.tile([P, HEADS], F32, tag="aj")
            nc.tensor.matmul(paj[:], lhsT=hT, rhs=vj_sb[:],
                             start=True, stop=True)
            w = sb.tile([P, HEADS], MMDT, tag=tag + "w")
            nc.scalar.activation(w[:], paj[:], AF.Exp)
            msg = sb.tile([P, 512], MMDT, tag=tag + "ms")
            nc.vector.tensor_tensor(
                msg[:].rearrange("p (h c) -> p h c", h=HEADS),
                phs[:].rearrange("p (h c) -> p h c", h=HEADS),
                w[:].to_broadcast([P, HEADS, EMBED]),
                op=OP.mult)
            S = sb.tile([P, P], MMDT, tag=tag + "S")
            nc.vector.tensor_tensor(S[:], drw[:, t - t_lo:t - t_lo + 1]
                                    .to_broadcast([P, P]), iota[:],
                                    op=OP.is_equal)
            nc.tensor.matmul(pM[:], lhsT=cast(S[:]), rhs=cast(msg[:]),
                             start=(t == t_lo), stop=(t == t_hi - 1))
            nc.tensor.matmul(pW[:], lhsT=S[:], rhs=w[:],
                             start=(t == t_lo), stop=(t == t_hi - 1))

        # finalize window: out = elu(res + mean_h(M/(W+eps)))
        rec = sb.tile([P, HEADS], F32, tag=tag + "rc")
        nc.vector.tensor_scalar_add(rec[:], pW[:], EPS)
        nc.vector.reciprocal(rec[:], rec[:])
        t1 = sb.tile([P, 512], F32, tag=tag + "t1")
        nc.vector.tensor_tensor(
            t1[:].rearrange("p (h c) -> p h c", h=HEADS),
            pM[:].rearrange("p (h c) -> p h c", h=HEADS),
            rec[:].to_broadcast([P, HEADS, EMBED]), op=OP.mult)
        s2 = sb.tile([P, 256], F32, tag=tag + "s2")
        nc.vector.tensor_tensor(s2[:], t1[:, 0:256], t1[:, 256:512], op=OP.add)
        s3 = sb.tile([P, 128], F32, tag=tag + "s3")
        nc.vector.tensor_tensor(s3[:], s2[:, 0:128], s2[:, 128:256], op=OP.add)
        s4 = sb.tile([P, 64], F32, tag=tag + "s4")
        nc.vector.tensor_tensor(s4[:], s3[:, 0:64], s3[:, 64:128], op=OP.add)
        res = sb.tile([P, EMBED], F32, tag=tag + "re")
        nc.sync.dma_start(res[:], res_dram[base:base + P, :])
        x = sb.tile([P, EMBED], F32, tag=tag + "x")
        nc.vector.tensor_scalar(x[:], s4[:], 0.125, None, op0=OP.mult)
        nc.vector.tensor_tensor(x[:], x[:], res[:], op=OP.add)
        o = sb.tile([P, EMBED], F32, tag=tag + "o")
        _elu(nc, sb, o[:], x[:], tag + "fe")
        for od in out_drams:
            nc.sync.dma_start(od[base:base + rows, :], o[:rows, :])


def build_program(p1, p2, use_f32r=True):
    nc = bacc.Bacc("TRN2", target_bir_lowering=False, num_devices=N_CORES)
    E1, E2 = p1["E_pad"], p2["E_pad"]
    nt1, nt2 = p1["tiles_total"], p2["tiles_total"]
    Dk1, Dk2 = p1["Dk"], p2["Dk"]
    nw1, nw2 = p1["n_win"], p2["n_win"]

    din = dict(
        d1_featT=nc.dram_tensor("d1_featT", [4, E1], MMDT, kind="ExternalInput"),
        d1_dstrel=nc.dram_tensor("d1_dstrel", [P, nt1], F32, kind="ExternalInput"),
        d2_srcidx=nc.dram_tensor("d2_srcidx", [P, nt2], I32, kind="ExternalInput"),
        d2_dstrel=nc.dram_tensor("d2_dstrel", [P, nt2], F32, kind="ExternalInput"),
        mc_slice=nc.dram_tensor("mc_slice", [nw1 * P, 2], F32, kind="ExternalInput"),
        op_slice=nc.dram_tensor("op_slice", [nw2 * P, 4], F32, kind="ExternalInput"),
        W_opT=nc.dram_tensor("W_opT", [4, EMBED], MMDT, kind="ExternalInput"),
        W_mcT=nc.dram_tensor("W_mcT", [2, EMBED], MMDT, kind="ExternalInput"),
        W_omT=nc.dram_tensor("W_omT", [EMBED, 512], MMDT, kind="ExternalInput"),
        W_moT=nc.dram_tensor("W_moT", [EMBED, 512], MMDT, kind="ExternalInput"),
        vj1=nc.dram_tensor("vj1", [EMBED, HEADS], MMDT, kind="ExternalInput"),
        vj2=nc.dram_tensor("vj2", [EMBED, HEADS], MMDT, kind="ExternalInput"),
        iota=nc.dram_tensor("iota", [P, P], F32, kind="ExternalInput"),
    )
    h_mc_out = nc.dram_tensor("h_mc_out", [Dk1, EMBED], F32, kind="ExternalOutput")
    h_op_out = nc.dram_tensor("h_op_out", [Dk2, EMBED], F32, kind="ExternalOutput")
    h_mc_res = nc.dram_tensor("h_mc_res", [nw1 * P, EMBED], F32, kind="Internal")
    h_op_res = nc.dram_tensor("h_op_res", [nw2 * P, EMBED], F32, kind="Internal")
    cc_in = nc.dram_tensor("cc_in", [Dk1, EMBED], F32, kind="Internal")
    cc_out = nc.dram_tensor("cc_out", [Dk1 * N_CORES, EMBED], F32,
                            kind="Internal", addr_space="Shared")

    with ExitStack() as ctx:
        tc = ctx.enter_context(tile.TileContext(nc))
        sb = ctx.enter_context(tc.tile_pool(name="sb", bufs=3))
        wpool = ctx.enter_context(tc.tile_pool(name="wp", bufs=1))
        psA = ctx.enter_context(tc.tile_pool(name="psA", bufs=2, space="PSUM"))
        psB = ctx.enter_context(tc.tile_pool(name="psB", bufs=1, space="PSUM"))

        ident = wpool.tile([P, P], F32, tag="ident")
        make_identity(nc, ident[:])
        iota = wpool.tile([P, P], F32, tag="iota")
        nc.sync.dma_start(iota[:], din["iota"][:, :])

        def _load_const(name, shape):
            t = wpool.tile(shape, MMDT, tag=name)
            nc.sync.dma_start(t[:], din[name][:, :])
            return t

        W_opT_sb = _load_const("W_opT", [4, EMBED])
        W_mcT_sb = _load_const("W_mcT", [2, EMBED])
        W_omT_sb = _load_const("W_omT", [EMBED, 512])
        W_moT_sb = _load_const("W_moT", [EMBED, 512])
        vj1_sb = _load_const("vj1", [EMBED, HEADS])
        vj2_sb = _load_const("vj2", [EMBED, HEADS])

        _residual_precompute(nc, sb, psA, psB, din["mc_slice"], W_mcT_sb, 2,
                             nw1, h_mc_res, ident, "rm")
        _residual_precompute(nc, sb, psA, psB, din["op_slice"], W_opT_sb, 4,
                             nw2, h_op_res, ident, "ro")

        _gat_direction(nc, tc, sb, psA, psB, p1,
                       d1_featT=din["d1_featT"], srcidx_dram=None,
                       gather_src_dram=None, dstrel_dram=din["d1_dstrel"],
                       WT_sb={"W_feat": W_opT_sb, "W_big": W_omT_sb},
                       vj_sb=vj1_sb, res_dram=h_mc_res,
                       out_drams=[h_mc_out, cc_in], ident=ident, iota=iota,
                       use_f32r=use_f32r, tag="a")

        tc.strict_bb_all_engine_barrier()
        nc.gpsimd.collective_compute(
            "AllGather", OP.bypass,
            replica_groups=[list(range(N_CORES))],
            ins=[cc_in[:, :]], outs=[cc_out[:, :]])
        tc.strict_bb_all_engine_barrier()

        _gat_direction(nc, tc, sb, psA, psB, p2,
                       d1_featT=None, srcidx_dram=din["d2_srcidx"],
                       gather_src_dram=cc_out, dstrel_dram=din["d2_dstrel"],
                       WT_sb={"W_big": W_moT_sb}, vj_sb=vj2_sb,
                       res_dram=h_op_res, out_drams=[h_op_out], ident=ident,
                       iota=iota, use_f32r=use_f32r, tag="b")
    if not nc.is_finalized():
        nc.finalize()
    return nc


# ------------------------------------------------------------------- driver
def kernel(op_nodes, mc_nodes, edge_index_om, W_op, b_op, W_mc, b_mc,
           W_om, att_om, W_mo, att_mo):
    global LAST_EXEC_NS
    op_nodes = np.asarray(op_nodes, dtype=np.float32)
    mc_nodes = np.asarray(mc_nodes, dtype=np.float32)
    ei = np.asarray(edge_index_om)
    src, dst = ei[0].astype(np.int64), ei[1].astype(np.int64)
    n_op, n_mc = op_nodes.shape[0], mc_nodes.shape[0]

    p1 = build_dir_plan(src, dst, n_mc, feats=op_nodes)
    p2 = build_dir_plan(dst, src, n_op)

    nc = build_program(p1, p2, use_f32r=False)

    import ml_dtypes
    mmnp = ml_dtypes.bfloat16 if USE_BF16 else np.float32
    iota = np.broadcast_to(np.arange(P, dtype=np.float32), (P, P)).copy()
    nw1, nw2 = p1["n_win"], p2["n_win"]
    in_maps = []
    for k in range(N_CORES):
        mc_sl = np.zeros((nw1 * P, 2), np.float32)
        mc_sl[:p1["Dk"]] = mc_nodes[k * p1["Dk"]:(k + 1) * p1["Dk"]]
        op_sl = np.zeros((nw2 * P, 4), np.float32)
        op_sl[:p2["Dk"]] = op_nodes[k * p2["Dk"]:(k + 1) * p2["Dk"]]
        in_maps.append(dict(
            d1_featT=p1["featT"][k].astype(mmnp),
            d1_dstrel=p1["dstrel_c"][k],
            d2_srcidx=p2["srcidx_c"][k],
            d2_dstrel=p2["dstrel_c"][k],
            mc_slice=mc_sl, op_slice=op_sl,
            W_opT=np.ascontiguousarray(W_op.T).astype(mmnp),
            W_mcT=np.ascontiguousarray(W_mc.T).astype(mmnp),
            W_omT=np.ascontiguousarray(W_om.T).astype(mmnp),
            W_moT=np.ascontiguousarray(W_mo.T).astype(mmnp),
            vj1=_vjT(np.asarray(att_om, np.float32), np.asarray(W_om, np.float32)).astype(mmnp),
            vj2=_vjT(np.asarray(att_mo, np.float32), np.asarray(W_mo, np.float32)).astype(mmnp),
            iota=iota,
        ))

    trace = bool(os.environ.get("KERNEL_TRACE"))
    try:
        res = run_bass_kernel_spmd(nc, in_maps, list(range(N_CORES)), trace=trace)
    except ModuleNotFoundError:
        res = run_bass_kernel_spmd(nc, in_maps, list(range(N_CORES)), trace=False)
    LAST_EXEC_NS = res.exec_time_ns
    if res.profile_json:
        with open("/root/problem/profile.json", "w") as f:
            f.write(res.profile_json)
    if os.environ.get("KERNEL_TIME_RERUN"):
        import time as _time
        t0 = _time.time()
        run_bass_kernel_spmd(nc, in_maps, list(range(N_CORES)), trace=False)
        LAST_EXEC_NS = LAST_EXEC_NS or int((_time.time() - t0) * 1e9)
    h_mc = np.concatenate([r["h_mc_out"] for r in res.results], axis=0)
    h_op = np.concatenate([r["h_op_out"] for r in res.results], axis=0)
    return (h_op, h_mc)
